# revision 11
# baseline (speedup 1.0000x reference)
"""Cumulative link (ordinal) loss on 8 Trainium2 NeuronCores.

Reference: p_i = sigmoid(hi_i - x_i) - sigmoid(lo_i - x_i) with per-label
thresholds hi = [0,1,2,3,+inf][l], lo = [-inf,0,1,2,3][l];
loss = mean_i -ln(p_i + eps).

Device formulation (exp/ln only -- both live in the single ACT table set
`natural_log_exp_and_others`, so the activation table loads exactly once;
sigmoid and ln never share a set, which would cost ~1.4 us per switch):

    h   = x - l                       (fp16 TT, 2x)
    u   = exp(h)                      (ACT)
    m14 = [l <= 3], m1e = e*[l >= 1]  (fp16 TS chains, 4x)
    uu  = [u*m14 | u*m1e]             (fp16 TT x2, adjacent halves)
    S_ln  += sum ln(uu + 1)           (one ACT pass over both halves, accum)
    mh  = m1e * h                     (GpSimd TT; DVE fallback)
    PE ones-matmuls into PSUM:  sum(m14), sum(m1e), sum(mh)

Identity (C = ln(e-1), g = l - x = -h):
    -ln p_i = ln(u*[l<=3] + 1) + ln(e*u*[l>=1] + 1)
              - [l>=1]*h - C*[l>=1] + (C-1)*[l>=4]
host combine:
    loss = (S_ln - S_mh/e - C*S_m1e/e + (C-1)*(B - S_m14)) / B
(l=0: ln(1+e^x); l=4: softplus(x-3); interior: g - C + softplus(-g)
 + softplus(1-g). The reference's +eps shifts it <=5e-5 relative.)

Why this shape (hardware findings from this session):
  * int64->fp16 SWDGE cast-DMA hard-crashes the device (NRT unrecoverable)
    -> labels stream as raw int32 pairs on the sync HWDGE queue, low words
    extracted by a DVE strided copy.
  * tensor_scalar accum_out silently writes 0; tensor_tensor_reduce
    crashes at execution -> every reduction rides the (proven) ACT
    activation accumulator or TensorE ones-matmuls into PSUM.
  * All input DMAs are issued up-front (logits f32->fp16 on the SWDGE
    queue, labels on HWDGE); every buffer is resident so nothing waits on
    pool reuse. HBM floor: 12.58 MB/core at ~358 GB/s ~= 35 us.
  * Chunk widths ramp 512->2048->256: first compute starts ~3 us into the
    stream, and the post-stream tail is only a ~256-col pipeline.

Sharding: pure data parallel, 1/8 of the batch per core, [128 x 8192].
"""

import numpy as np

B_TOTAL = 8388608
N_CORES = 8
P = 128
SHARD = B_TOTAL // N_CORES          # 1048576 per core
M = SHARD // P                      # 8192 free-dim columns per core
WIDTHS = (256, 768, 1536, 2048, 1792, 1024, 512, 256)
NCH = len(WIDTHS)
E = 2.718281828459045
C = 0.5413248546129181              # ln(e - 1)
RED_W = 512                         # PSUM accumulation width per stream
MH_ON_GPSIMD = False

_NC = None


def _build_nc():
    import concourse.bacc as bacc
    import concourse.mybir as mybir
    from concourse import tile
    from concourse.hw_specs import get_activation_tables

    f32 = mybir.dt.float32
    f16 = mybir.dt.float16
    i32 = mybir.dt.int32
    Alu = mybir.AluOpType
    Act = mybir.ActivationFunctionType

    assert sum(WIDTHS) == M

    nc = bacc.Bacc("TRN2", target_bir_lowering=False, debug=False,
                   enable_asserts=False)

    # Steer table selection: leave Exp/Ln only in the one set that has both.
    tabs = get_activation_tables(nc.m.arch)
    assert "natural_log_exp_and_others" in tabs
    for name, fns in tabs.items():
        if name != "natural_log_exp_and_others":
            fns.discard(Act.Exp)
            fns.discard(Act.Ln)

    x_dram = nc.dram_tensor("logits", (P, M), f32, kind="ExternalInput")
    l_dram = nc.dram_tensor("labels", (P, 2 * M), i32, kind="ExternalInput")
    o_dram = nc.dram_tensor("out", (P, NCH), f32, kind="ExternalOutput")
    r_dram = nc.dram_tensor("red", (1, 3 * RED_W), f32, kind="ExternalOutput")

    offs = [0]
    for w in WIDTHS:
        offs.append(offs[-1] + w)

    with tile.TileContext(nc) as tc:
        with tc.tile_pool(name="p", bufs=1) as pp, \
             tc.psum_pool(name="ps", bufs=1) as psp:
            acc = pp.tile([P, NCH], f32, tag="acc")
            # [P, 2] not [P, 1]: a 2-byte tile would knock every later f16
            # tile off 4-byte alignment, demoting all DVE TTs from 2x to 1x.
            ones = pp.tile([P, 2], f16, tag="ones")
            nc.vector.memset(ones[:], 1.0)
            ps = psp.tile([1, 3, RED_W], f32, tag="ps")

            x16s, l32s = [], []
            for k, W in enumerate(WIDTHS):
                a, b = offs[k], offs[k + 1]
                x16 = pp.tile([P, W], f16, tag=f"x{k}")
                l32 = pp.tile([P, W, 2], i32, tag=f"l{k}")
                nc.gpsimd.dma_start(out=x16[:], in_=x_dram[:, a:b])
                nc.sync.dma_start(out=l32[:], in_=l_dram[:, 2 * a:2 * b])
                x16s.append(x16)
                l32s.append(l32)

            nmm = [0, 0, 0]
            tot = [M // RED_W + (1 if M % RED_W else 0)] * 3
            nslices = sum((W + RED_W - 1) // RED_W for W in WIDTHS)
            tot = [nslices] * 3

            for k, W in enumerate(WIDTHS):
                x16, l32 = x16s[k], l32s[k]
                lev = pp.tile([P, W], f16, tag=f"lev{k}")
                h = pp.tile([P, W], f16, tag=f"h{k}")
                u = pp.tile([P, W], f16, tag=f"u{k}")
                uu = pp.tile([P, 2 * W], f16, tag=f"uu{k}")
                m14 = pp.tile([P, W], f16, tag=f"m14_{k}")
                m1e = pp.tile([P, W], f16, tag=f"m1e_{k}")
                mh = x16         # x16 is dead after the h TT; reuse its slot

                nc.vector.tensor_copy(out=lev[:], in_=l32[:, :, 0])
                nc.vector.tensor_tensor(out=h[:], in0=x16[:], in1=lev[:],
                                        op=Alu.subtract)
                nc.scalar.activation(u[:], h[:], Act.Exp)
                nc.vector.tensor_scalar(out=m14[:], in0=lev[:], scalar1=3.5,
                                        scalar2=None, op0=Alu.is_le)
                nc.vector.tensor_scalar(out=m1e[:], in0=lev[:], scalar1=0.5,
                                        scalar2=E, op0=Alu.is_ge,
                                        op1=Alu.mult)
                if MH_ON_GPSIMD:
                    nc.gpsimd.tensor_tensor(out=mh[:], in0=m1e[:], in1=h[:],
                                            op=Alu.mult)
                else:
                    nc.vector.tensor_tensor(out=mh[:], in0=m1e[:], in1=h[:],
                                            op=Alu.mult)
                nc.vector.tensor_tensor(out=uu[:, 0:W], in0=u[:], in1=m14[:],
                                        op=Alu.mult)
                nc.vector.tensor_tensor(out=uu[:, W:2 * W], in0=u[:],
                                        in1=m1e[:], op=Alu.mult)
                nc.scalar.activation(uu[:], uu[:], Act.Ln, bias=1.0,
                                     accum_out=acc[:, k:k + 1])

                for s0 in range(0, W, RED_W):
                    w = min(RED_W, W - s0)
                    for j, src in enumerate((m14, m1e, mh)):
                        nc.tensor.matmul(ps[:, j, 0:w], ones[:, 0:1],
                                         src[:, s0:s0 + w],
                                         start=(nmm[j] == 0),
                                         stop=(nmm[j] == tot[j] - 1),
                                         skip_group_check=True)
                        nmm[j] += 1

            # stage the PSUM row in l32s[1]'s dead slot (8 KB/partition)
            fin = l32s[1][:].rearrange("p a b -> p (a b)").bitcast(f32)[0:1, 0:3 * RED_W]
            nc.vector.tensor_copy(out=fin, in_=ps[:].rearrange("p a b -> p (a b)"))
            nc.sync.dma_start(out=o_dram[:], in_=acc[:])
            nc.sync.dma_start(out=r_dram[:], in_=fin)

    nc.compile()
    return nc


def get_nc():
    global _NC
    if _NC is None:
        _NC = _build_nc()
    return _NC


def make_in_maps(logits, labels):
    x = np.ascontiguousarray(np.asarray(logits, dtype=np.float32)).reshape(B_TOTAL)
    lab = np.asarray(labels)
    if lab.dtype != np.int64:
        lab = lab.astype(np.int64)
    lab = np.ascontiguousarray(lab).reshape(B_TOTAL)
    in_maps = []
    for c in range(N_CORES):
        xs = x[c * SHARD:(c + 1) * SHARD].reshape(P, M)
        ls = lab[c * SHARD:(c + 1) * SHARD].view(np.int32).reshape(P, 2 * M)
        in_maps.append({"logits": xs, "labels": ls})
    return in_maps


def run(logits, labels, trace=False):
    """Returns (loss_scalar_f32, BassKernelResults)."""
    from concourse.bass_utils import run_bass_kernel_spmd

    nc = get_nc()
    in_maps = make_in_maps(logits, labels)
    res = run_bass_kernel_spmd(
        nc, in_maps, core_ids=list(range(N_CORES)), trace=trace
    )
    s_ln = s_m14 = s_m1e = s_mh = 0.0
    for r in res.results:
        s_ln += r["out"].astype(np.float64).sum()
        red = r["red"].astype(np.float64).reshape(3, RED_W)
        s_m14 += red[0].sum()
        s_m1e += red[1].sum()
        s_mh += red[2].sum()
    n1 = s_m1e / E
    n4 = B_TOTAL - s_m14
    total = s_ln - s_mh / E - C * n1 + (C - 1.0) * n4
    loss = np.float32(total / B_TOTAL)
    return np.asarray(loss), res


def kernel(logits, labels):
    out, _ = run(logits, labels, trace=False)
    return out


# revision 12
# speedup vs baseline: 1.0730x; 1.0730x over previous
"""Cumulative link (ordinal) loss on 8 Trainium2 NeuronCores.

Reference: p_i = sigmoid(hi_i - x_i) - sigmoid(lo_i - x_i) with per-label
thresholds hi = [0,1,2,3,+inf][l], lo = [-inf,0,1,2,3][l];
loss = mean_i -ln(p_i + eps).

Device formulation (exp/ln only -- both live in the single ACT table set
`natural_log_exp_and_others`, so the activation table loads exactly once;
sigmoid and ln never share a set, which would cost ~1.4 us per switch):

    h   = x - l                       (fp16 TT, 2x)
    u   = exp(h)                      (ACT)
    m14 = [l <= 3], m1e = e*[l >= 1]  (fp16 TS chains, 4x)
    uu  = [u*m14 | u*m1e]             (fp16 TT x2, adjacent halves)
    S_ln  += sum ln(uu + 1)           (one ACT pass over both halves, accum)
    mh  = m1e * h                     (GpSimd TT; DVE fallback)
    PE ones-matmuls into PSUM:  sum(m14), sum(m1e), sum(mh)

Identity (C = ln(e-1), g = l - x = -h):
    -ln p_i = ln(u*[l<=3] + 1) + ln(e*u*[l>=1] + 1)
              - [l>=1]*h - C*[l>=1] + (C-1)*[l>=4]
host combine:
    loss = (S_ln - S_mh/e - C*S_m1e/e + (C-1)*(B - S_m14)) / B
(l=0: ln(1+e^x); l=4: softplus(x-3); interior: g - C + softplus(-g)
 + softplus(1-g). The reference's +eps shifts it <=5e-5 relative.)

Why this shape (hardware findings from this session):
  * int64->fp16 SWDGE cast-DMA hard-crashes the device (NRT unrecoverable)
    -> labels stream as raw int32 pairs on the sync HWDGE queue, low words
    extracted by a DVE strided copy.
  * tensor_scalar accum_out silently writes 0; tensor_tensor_reduce
    crashes at execution -> every reduction rides the (proven) ACT
    activation accumulator or TensorE ones-matmuls into PSUM.
  * All input DMAs are issued up-front (logits f32->fp16 on the SWDGE
    queue, labels on HWDGE); every buffer is resident so nothing waits on
    pool reuse. HBM floor: 12.58 MB/core at ~358 GB/s ~= 35 us.
  * Chunk widths ramp 512->2048->256: first compute starts ~3 us into the
    stream, and the post-stream tail is only a ~256-col pipeline.

Sharding: pure data parallel, 1/8 of the batch per core, [128 x 8192].
"""

import numpy as np

B_TOTAL = 8388608
N_CORES = 8
P = 128
SHARD = B_TOTAL // N_CORES          # 1048576 per core
M = SHARD // P                      # 8192 free-dim columns per core
WIDTHS = (512, 1024, 1536, 2048, 1536, 768, 512, 256)
NCH = len(WIDTHS)
E = 2.718281828459045
C = 0.5413248546129181              # ln(e - 1)
RED_W = 512                         # PSUM accumulation width per stream
MH_ON_GPSIMD = True

_NC = None


def _build_nc():
    import concourse.bacc as bacc
    import concourse.mybir as mybir
    from concourse import tile
    from concourse.hw_specs import get_activation_tables

    f32 = mybir.dt.float32
    f16 = mybir.dt.float16
    i32 = mybir.dt.int32
    Alu = mybir.AluOpType
    Act = mybir.ActivationFunctionType

    assert sum(WIDTHS) == M

    nc = bacc.Bacc("TRN2", target_bir_lowering=False, debug=False,
                   enable_asserts=False)

    # Steer table selection: leave Exp/Ln only in the one set that has both.
    tabs = get_activation_tables(nc.m.arch)
    assert "natural_log_exp_and_others" in tabs
    for name, fns in tabs.items():
        if name != "natural_log_exp_and_others":
            fns.discard(Act.Exp)
            fns.discard(Act.Ln)

    x_dram = nc.dram_tensor("logits", (P, M), f32, kind="ExternalInput")
    l_dram = nc.dram_tensor("labels", (P, 2 * M), i32, kind="ExternalInput")
    o_dram = nc.dram_tensor("out", (P, NCH), f32, kind="ExternalOutput")
    r_dram = nc.dram_tensor("red", (1, 3 * RED_W), f32, kind="ExternalOutput")

    offs = [0]
    for w in WIDTHS:
        offs.append(offs[-1] + w)

    with tile.TileContext(nc) as tc:
        with tc.tile_pool(name="p", bufs=1) as pp, \
             tc.psum_pool(name="ps", bufs=1) as psp:
            acc = pp.tile([P, NCH], f32, tag="acc")
            # [P, 2] not [P, 1]: a 2-byte tile would knock every later f16
            # tile off 4-byte alignment, demoting all DVE TTs from 2x to 1x.
            ones = pp.tile([P, 2], f16, tag="ones")
            nc.vector.memset(ones[:], 1.0)
            ps = psp.tile([1, 3, RED_W], f32, tag="ps")

            x16s, l32s = [], []
            for k, W in enumerate(WIDTHS):
                a, b = offs[k], offs[k + 1]
                x16 = pp.tile([P, W], f16, tag=f"x{k}")
                l32 = pp.tile([P, W, 2], i32, tag=f"l{k}")
                nc.gpsimd.dma_start(out=x16[:], in_=x_dram[:, a:b])
                nc.sync.dma_start(out=l32[:], in_=l_dram[:, 2 * a:2 * b])
                x16s.append(x16)
                l32s.append(l32)

            nmm = [0, 0, 0]
            tot = [M // RED_W + (1 if M % RED_W else 0)] * 3
            nslices = sum((W + RED_W - 1) // RED_W for W in WIDTHS)
            tot = [nslices] * 3

            for k, W in enumerate(WIDTHS):
                x16, l32 = x16s[k], l32s[k]
                lev = pp.tile([P, W], f16, tag=f"lev{k}")
                h = pp.tile([P, W], f16, tag=f"h{k}")
                u = pp.tile([P, W], f16, tag=f"u{k}")
                uu = pp.tile([P, 2 * W], f16, tag=f"uu{k}")
                m14 = pp.tile([P, W], f16, tag=f"m14_{k}")
                m1e = pp.tile([P, W], f16, tag=f"m1e_{k}")
                mh = x16         # x16 is dead after the h TT; reuse its slot

                nc.vector.tensor_copy(out=lev[:], in_=l32[:, :, 0])
                nc.vector.tensor_tensor(out=h[:], in0=x16[:], in1=lev[:],
                                        op=Alu.subtract)
                nc.scalar.activation(u[:], h[:], Act.Exp)
                nc.vector.tensor_scalar(out=m14[:], in0=lev[:], scalar1=3.5,
                                        scalar2=None, op0=Alu.is_le)
                nc.vector.tensor_scalar(out=m1e[:], in0=lev[:], scalar1=0.5,
                                        scalar2=E, op0=Alu.is_ge,
                                        op1=Alu.mult)
                if MH_ON_GPSIMD:
                    nc.gpsimd.tensor_tensor(out=mh[:], in0=m1e[:], in1=h[:],
                                            op=Alu.mult)
                else:
                    nc.vector.tensor_tensor(out=mh[:], in0=m1e[:], in1=h[:],
                                            op=Alu.mult)
                nc.vector.tensor_tensor(out=uu[:, 0:W], in0=u[:], in1=m14[:],
                                        op=Alu.mult)
                nc.vector.tensor_tensor(out=uu[:, W:2 * W], in0=u[:],
                                        in1=m1e[:], op=Alu.mult)
                nc.scalar.activation(uu[:], uu[:], Act.Ln, bias=1.0,
                                     accum_out=acc[:, k:k + 1])

                for s0 in range(0, W, RED_W):
                    w = min(RED_W, W - s0)
                    for j, src in enumerate((m14, m1e, mh)):
                        nc.tensor.matmul(ps[:, j, 0:w], ones[:, 0:1],
                                         src[:, s0:s0 + w],
                                         start=(nmm[j] == 0),
                                         stop=(nmm[j] == tot[j] - 1),
                                         skip_group_check=True)
                        nmm[j] += 1

            # stage the PSUM row in l32s[1]'s dead slot (8 KB/partition)
            fin = l32s[1][:].rearrange("p a b -> p (a b)").bitcast(f32)[0:1, 0:3 * RED_W]
            nc.vector.tensor_copy(out=fin, in_=ps[:].rearrange("p a b -> p (a b)"))
            nc.sync.dma_start(out=o_dram[:], in_=acc[:])
            nc.sync.dma_start(out=r_dram[:], in_=fin)

    nc.compile()
    return nc


def get_nc():
    global _NC
    if _NC is None:
        _NC = _build_nc()
    return _NC


def make_in_maps(logits, labels):
    x = np.ascontiguousarray(np.asarray(logits, dtype=np.float32)).reshape(B_TOTAL)
    lab = np.asarray(labels)
    if lab.dtype != np.int64:
        lab = lab.astype(np.int64)
    lab = np.ascontiguousarray(lab).reshape(B_TOTAL)
    in_maps = []
    for c in range(N_CORES):
        xs = x[c * SHARD:(c + 1) * SHARD].reshape(P, M)
        ls = lab[c * SHARD:(c + 1) * SHARD].view(np.int32).reshape(P, 2 * M)
        in_maps.append({"logits": xs, "labels": ls})
    return in_maps


def run(logits, labels, trace=False):
    """Returns (loss_scalar_f32, BassKernelResults)."""
    from concourse.bass_utils import run_bass_kernel_spmd

    nc = get_nc()
    in_maps = make_in_maps(logits, labels)
    res = run_bass_kernel_spmd(
        nc, in_maps, core_ids=list(range(N_CORES)), trace=trace
    )
    s_ln = s_m14 = s_m1e = s_mh = 0.0
    for r in res.results:
        s_ln += r["out"].astype(np.float64).sum()
        red = r["red"].astype(np.float64).reshape(3, RED_W)
        s_m14 += red[0].sum()
        s_m1e += red[1].sum()
        s_mh += red[2].sum()
    n1 = s_m1e / E
    n4 = B_TOTAL - s_m14
    total = s_ln - s_mh / E - C * n1 + (C - 1.0) * n4
    loss = np.float32(total / B_TOTAL)
    return np.asarray(loss), res


def kernel(logits, labels):
    out, _ = run(logits, labels, trace=False)
    return out


# revision 13
# speedup vs baseline: 1.1169x; 1.0410x over previous
"""Cumulative link (ordinal) loss on 8 Trainium2 NeuronCores.

Reference: p_i = sigmoid(hi_i - x_i) - sigmoid(lo_i - x_i) with per-label
thresholds hi = [0,1,2,3,+inf][l], lo = [-inf,0,1,2,3][l];
loss = mean_i -ln(p_i + eps).

Device formulation (exp/ln only -- both live in the single ACT table set
`natural_log_exp_and_others`, so the activation table loads exactly once;
sigmoid and ln never share a set, which would cost ~1.4 us per switch):

    h   = x - l                       (fp16 TT, 2x)
    u   = exp(h)                      (ACT)
    m14 = [l <= 3], m1e = e*[l >= 1]  (fp16 TS chains, 4x)
    uu  = [u*m14 | u*m1e]             (fp16 TT x2, adjacent halves)
    S_ln  += sum ln(uu + 1)           (one ACT pass over both halves, accum)
    mh  = m1e * h                     (GpSimd TT; DVE fallback)
    PE ones-matmuls into PSUM:  sum(m14), sum(m1e), sum(mh)

Identity (C = ln(e-1), g = l - x = -h):
    -ln p_i = ln(u*[l<=3] + 1) + ln(e*u*[l>=1] + 1)
              - [l>=1]*h - C*[l>=1] + (C-1)*[l>=4]
host combine:
    loss = (S_ln - S_mh/e - C*S_m1e/e + (C-1)*(B - S_m14)) / B
(l=0: ln(1+e^x); l=4: softplus(x-3); interior: g - C + softplus(-g)
 + softplus(1-g). The reference's +eps shifts it <=5e-5 relative.)

Why this shape (hardware findings from this session):
  * int64->fp16 SWDGE cast-DMA hard-crashes the device (NRT unrecoverable)
    -> labels stream as raw int32 pairs on the sync HWDGE queue, low words
    extracted by a DVE strided copy.
  * tensor_scalar accum_out silently writes 0; tensor_tensor_reduce
    crashes at execution -> every reduction rides the (proven) ACT
    activation accumulator or TensorE ones-matmuls into PSUM.
  * All input DMAs are issued up-front (logits f32->fp16 on the SWDGE
    queue, labels on HWDGE); every buffer is resident so nothing waits on
    pool reuse. HBM floor: 12.58 MB/core at ~358 GB/s ~= 35 us.
  * Chunk widths ramp 512->2048->256: first compute starts ~3 us into the
    stream, and the post-stream tail is only a ~256-col pipeline.

Sharding: pure data parallel, 1/8 of the batch per core, [128 x 8192].
"""

import numpy as np

B_TOTAL = 8388608
N_CORES = 8
P = 128
SHARD = B_TOTAL // N_CORES          # 1048576 per core
M = SHARD // P                      # 8192 free-dim columns per core
WIDTHS = (512, 1024, 1536, 2048, 1536, 768, 512, 256)
NCH = len(WIDTHS)
E = 2.718281828459045
C = 0.5413248546129181              # ln(e - 1)
RED_W = 512                         # PSUM accumulation width per stream
MH_ON_GPSIMD = False

_NC = None


def _build_nc():
    import concourse.bacc as bacc
    import concourse.mybir as mybir
    from concourse import tile
    from concourse.hw_specs import get_activation_tables

    f32 = mybir.dt.float32
    f16 = mybir.dt.float16
    i32 = mybir.dt.int32
    Alu = mybir.AluOpType
    Act = mybir.ActivationFunctionType

    assert sum(WIDTHS) == M

    nc = bacc.Bacc("TRN2", target_bir_lowering=False, debug=False,
                   enable_asserts=False)

    # Steer table selection: leave Exp/Ln only in the one set that has both.
    tabs = get_activation_tables(nc.m.arch)
    assert "natural_log_exp_and_others" in tabs
    for name, fns in tabs.items():
        if name != "natural_log_exp_and_others":
            fns.discard(Act.Exp)
            fns.discard(Act.Ln)

    x_dram = nc.dram_tensor("logits", (P, M), f32, kind="ExternalInput")
    l_dram = nc.dram_tensor("labels", (P, 2 * M), i32, kind="ExternalInput")
    o_dram = nc.dram_tensor("out", (P, NCH), f32, kind="ExternalOutput")
    r_dram = nc.dram_tensor("red", (1, 3 * RED_W), f32, kind="ExternalOutput")

    offs = [0]
    for w in WIDTHS:
        offs.append(offs[-1] + w)

    with tile.TileContext(nc) as tc:
        with tc.tile_pool(name="p", bufs=1) as pp, \
             tc.psum_pool(name="ps", bufs=1) as psp:
            acc = pp.tile([P, NCH], f32, tag="acc")
            # [P, 2] not [P, 1]: a 2-byte tile would knock every later f16
            # tile off 4-byte alignment, demoting all DVE TTs from 2x to 1x.
            ones = pp.tile([P, 2], f16, tag="ones")
            nc.vector.memset(ones[:], 1.0)
            ps = psp.tile([1, 3, RED_W], f32, tag="ps")

            x16s, l32s = [], []
            for k, W in enumerate(WIDTHS):
                a, b = offs[k], offs[k + 1]
                x16 = pp.tile([P, W], f16, tag=f"x{k}")
                l32 = pp.tile([P, W, 2], i32, tag=f"l{k}")
                nc.gpsimd.dma_start(out=x16[:], in_=x_dram[:, a:b])
                nc.sync.dma_start(out=l32[:], in_=l_dram[:, 2 * a:2 * b])
                x16s.append(x16)
                l32s.append(l32)

            nmm = [0, 0, 0]
            tot = [M // RED_W + (1 if M % RED_W else 0)] * 3
            nslices = sum((W + RED_W - 1) // RED_W for W in WIDTHS)
            tot = [nslices] * 3

            for k, W in enumerate(WIDTHS):
                x16, l32 = x16s[k], l32s[k]
                lev = pp.tile([P, W], f16, tag=f"lev{k}")
                h = pp.tile([P, W], f16, tag=f"h{k}")
                u = pp.tile([P, W], f16, tag=f"u{k}")
                uu = pp.tile([P, 2 * W], f16, tag=f"uu{k}")
                m14 = pp.tile([P, W], f16, tag=f"m14_{k}")
                m1e = pp.tile([P, W], f16, tag=f"m1e_{k}")
                mh = x16         # x16 is dead after the h TT; reuse its slot

                nc.vector.tensor_copy(out=lev[:], in_=l32[:, :, 0])
                nc.vector.tensor_tensor(out=h[:], in0=x16[:], in1=lev[:],
                                        op=Alu.subtract)
                nc.scalar.activation(u[:], h[:], Act.Exp)
                nc.vector.tensor_scalar(out=m14[:], in0=lev[:], scalar1=3.5,
                                        scalar2=None, op0=Alu.is_le)
                nc.vector.tensor_scalar(out=m1e[:], in0=lev[:], scalar1=0.5,
                                        scalar2=E, op0=Alu.is_ge,
                                        op1=Alu.mult)
                if MH_ON_GPSIMD:
                    nc.gpsimd.tensor_tensor(out=mh[:], in0=m1e[:], in1=h[:],
                                            op=Alu.mult)
                else:
                    nc.vector.tensor_tensor(out=mh[:], in0=m1e[:], in1=h[:],
                                            op=Alu.mult)
                nc.vector.tensor_tensor(out=uu[:, 0:W], in0=u[:], in1=m14[:],
                                        op=Alu.mult)
                nc.vector.tensor_tensor(out=uu[:, W:2 * W], in0=u[:],
                                        in1=m1e[:], op=Alu.mult)
                nc.scalar.activation(uu[:], uu[:], Act.Ln, bias=1.0,
                                     accum_out=acc[:, k:k + 1])

                for s0 in range(0, W, RED_W):
                    w = min(RED_W, W - s0)
                    for j, src in enumerate((m14, m1e, mh)):
                        nc.tensor.matmul(ps[:, j, 0:w], ones[:, 0:1],
                                         src[:, s0:s0 + w],
                                         start=(nmm[j] == 0),
                                         stop=(nmm[j] == tot[j] - 1),
                                         skip_group_check=True)
                        nmm[j] += 1

            # stage the PSUM row in l32s[1]'s dead slot (8 KB/partition)
            fin = l32s[1][:].rearrange("p a b -> p (a b)").bitcast(f32)[0:1, 0:3 * RED_W]
            nc.vector.tensor_copy(out=fin, in_=ps[:].rearrange("p a b -> p (a b)"))
            nc.sync.dma_start(out=o_dram[:], in_=acc[:])
            nc.sync.dma_start(out=r_dram[:], in_=fin)

    nc.compile()
    return nc


def get_nc():
    global _NC
    if _NC is None:
        _NC = _build_nc()
    return _NC


def make_in_maps(logits, labels):
    x = np.ascontiguousarray(np.asarray(logits, dtype=np.float32)).reshape(B_TOTAL)
    lab = np.asarray(labels)
    if lab.dtype != np.int64:
        lab = lab.astype(np.int64)
    lab = np.ascontiguousarray(lab).reshape(B_TOTAL)
    in_maps = []
    for c in range(N_CORES):
        xs = x[c * SHARD:(c + 1) * SHARD].reshape(P, M)
        ls = lab[c * SHARD:(c + 1) * SHARD].view(np.int32).reshape(P, 2 * M)
        in_maps.append({"logits": xs, "labels": ls})
    return in_maps


def run(logits, labels, trace=False):
    """Returns (loss_scalar_f32, BassKernelResults)."""
    from concourse.bass_utils import run_bass_kernel_spmd

    nc = get_nc()
    in_maps = make_in_maps(logits, labels)
    res = run_bass_kernel_spmd(
        nc, in_maps, core_ids=list(range(N_CORES)), trace=trace
    )
    s_ln = s_m14 = s_m1e = s_mh = 0.0
    for r in res.results:
        s_ln += r["out"].astype(np.float64).sum()
        red = r["red"].astype(np.float64).reshape(3, RED_W)
        s_m14 += red[0].sum()
        s_m1e += red[1].sum()
        s_mh += red[2].sum()
    n1 = s_m1e / E
    n4 = B_TOTAL - s_m14
    total = s_ln - s_mh / E - C * n1 + (C - 1.0) * n4
    loss = np.float32(total / B_TOTAL)
    return np.asarray(loss), res


def kernel(logits, labels):
    out, _ = run(logits, labels, trace=False)
    return out
